# revision 36
# baseline (speedup 1.0000x reference)
"""GAT layer (BatchNorm -> GATConv -> head-mean -> ELU -> per-graph Conv1d)
on 8 Trainium2 NeuronCores via Bass/Tile.

Sharding: graphs (nodes + their incoming edges) are sharded across the 8
cores by destination node.  The host does index manipulation only
(sharding / sorting / padding / fancy-indexing of raw input rows); every
FLOP on tensor data happens on-device.

Per core:
  1. BN statistics over the full x (device), fused scale/shift + attention
     row vectors into a broadcast table,
  2. edges are pre-sorted by destination in-degree rank on the host so the
     per-destination segment softmax accumulates as dense "round" adds (no
     scatter); the per-edge source operand stream is the host-sharded
     raw x rows (+ a dummy marker lane), normalized and projected to
     attention logits on-device,
  3. exp / weighted payload accumulation in a [128, rank-block, 24] f32
     accumulator; finalized rank blocks are normalized, projected
     (head-mean folded), biased, ELU'd and spilled inline,
  4. un-permute node order (batched dma_gather from the spill table) and
     run the per-graph Conv1d as chunked bf16 matmuls.
"""

import sys

sys.path.insert(0, "/opt/trn_rl_repo")

import numpy as np
from contextlib import ExitStack

import concourse.bass as bass
import concourse.bacc as bacc
import concourse.tile as tile
from concourse import mybir
from concourse.masks import make_identity
from concourse.bass_utils import run_bass_kernel_spmd

F32 = mybir.dt.float32
BF16 = mybir.dt.bfloat16
I16 = mybir.dt.int16
AF = mybir.ActivationFunctionType
OP = mybir.AluOpType

N = 190464
FIN = 5
FS = 6             # streamed operand words per edge: [x(5) | marker]
H = 4
C = 24
NPG = 186          # nodes per graph
B = 1024           # graphs
NCORES = 8
GPC = B // NCORES  # 128 graphs per core
NL = N // NCORES   # 23808 local nodes per core
NBLK = NL // 128   # 186 rank blocks of 128
MB = N // 128      # 1488 nodes per partition in the flat x layout
KCONV = 62
COUT = 8
TOUT = NPG - KCONV + 1   # 125
PADG = 192               # per-graph padded length (conv shift head-room)
NPOS = GPC * PADG        # 24576 padded node positions per core
NPT = NPOS // 128        # 192 position tiles
G_CH = 128               # max edge tiles per chunk
NU = 24                  # accum payload: 20 = w_h * xn_f, 4 = w_h
EPS = 1e-5
DUMMY_ASRC = -400.0      # exp(0.2 * (DUMMY_ASRC + adst)) ~ 1e-35, in ACT range
EL = 64                  # f32 words per spill row (legacy)
ELB = 128                # bf16 words per spill row (256B, transpose-gather)


# --------------------------------------------------------------------------
# host-side sharding / ordering (pure index manipulation)
# --------------------------------------------------------------------------
def _wrap16(vals16):
    """Wrap a linear int16 index stream for dma_gather: position i lives at
    [i % 16, i // 16]; replicate the 16-partition block to all 128."""
    cols = vals16.size // 16
    w = vals16.reshape(cols, 16).T
    return np.ascontiguousarray(np.tile(w, (8, 1)))


def _host_prep(x, edge_index):
    src_g = np.asarray(edge_index[0], dtype=np.int64)
    dst_g = np.asarray(edge_index[1], dtype=np.int64)
    cores = []
    maxdeg = 0
    for k in range(NCORES):
        lo = k * NL
        m = (dst_g >= lo) & (dst_g < lo + NL)
        # self-loops are handled by a dense on-device pass, not as edges
        es = src_g[m]
        ed = dst_g[m] - lo
        deg = np.bincount(ed, minlength=NL)
        node_of_rank = np.argsort(-deg, kind="stable")
        rank_of_node = np.empty(NL, dtype=np.int64)
        rank_of_node[node_of_rank] = np.arange(NL)
        r_e = rank_of_node[ed]
        perm = np.argsort(r_e, kind="stable")
        es_s = es[perm]
        r_s = r_e[perm]
        cnt = deg[node_of_rank]          # per-rank degree, descending
        starts = np.zeros(NL, dtype=np.int64)
        starts[1:] = np.cumsum(cnt)[:-1]
        j_s = np.arange(es_s.size, dtype=np.int64) - starts[r_s]
        maxdeg = max(maxdeg, int(cnt[0]))
        cores.append(dict(es_s=es_s, r_s=r_s, j_s=j_s, cnt=cnt,
                          rank_of_node=rank_of_node,
                          node_of_rank=node_of_rank))

    # global (SPMD-identical) round sizes: K_j = #nodes with deg > j
    kmax = np.zeros(maxdeg, dtype=np.int64)
    for c in cores:
        kj = np.searchsorted(-c["cnt"], -np.arange(maxdeg), side="left")
        kmax = np.maximum(kmax, kj)
    r_tiles = (kmax + 127) // 128            # tiles per round
    r_edges = r_tiles * 128
    round_base = np.zeros(maxdeg + 1, dtype=np.int64)
    round_base[1:] = np.cumsum(r_edges)
    e_pad = int(round_base[-1])
    nt_total = e_pad // 128

    # chunk schedule (identical across cores): (t0, c0, nt, fin_lo, fin_hi)
    # where [fin_lo, fin_hi) are the rank blocks finalized after this chunk
    # (no later round touches them -> normalize/project/spill them inline).
    chunks = []
    t0 = 0
    for j in range(maxdeg):
        rem = int(r_tiles[j])
        c0 = 0
        while rem:
            nt = min(G_CH, rem)
            last_of_round = (rem == nt)
            if last_of_round:
                hi = int(r_tiles[j])
                lo = int(r_tiles[j + 1]) if j + 1 < maxdeg else 0
            else:
                lo = hi = 0
            chunks.append((t0, c0, nt, lo, hi))
            t0 += nt
            c0 += nt
            rem -= nt
    assert t0 == nt_total
    fin0 = int(r_tiles[0]) if maxdeg else 0   # blocks never touched by rounds

    per_core = []
    for c in cores:
        stream = np.full(e_pad, N, dtype=np.int64)    # N = dummy marker
        pos = round_base[c["j_s"]] + c["r_s"]
        stream[pos] = c["es_s"]
        real = stream < N
        # per-position raw operand rows: [x(5) | marker]; dummy rows get
        # marker DUMMY_ASRC so their exp-weight underflows to ~0
        xs = np.zeros((e_pad, FS), dtype=np.float32)
        xs[real, 0:FIN] = x[stream[real]]
        xs[~real, FIN] = DUMMY_ASRC
        # position t*128+p lives at [p, t*FS : t*FS+FS]
        import ml_dtypes
        xs_sb = np.ascontiguousarray(
            xs.reshape(nt_total, 128, FS).transpose(1, 0, 2).reshape(
                128, nt_total * FS)).astype(ml_dtypes.bfloat16)

        gid = np.full(NPOS, NL, dtype=np.int64)       # NL = dummy zero row
        posg = np.arange(NPOS)
        g = posg // PADG
        s = posg % PADG
        real_g = s < NPG
        gid[real_g] = c["rank_of_node"][g[real_g] * NPG + s[real_g]]
        gidx16 = gid.astype(np.int16)

        per_core.append(dict(xs=xs_sb, gidx16=_wrap16(gidx16),
                             node_of_rank=c["node_of_rank"]))

    return per_core, chunks, nt_total, fin0


def _w3_layout(lin_w, gat_bias):
    """Pure layout: rows (h,f) = lin_w[h*24+c', f]; row 20 = gat_bias."""
    w3 = np.zeros((NU, C), dtype=np.float32)
    for h in range(H):
        w3[h * FIN:(h + 1) * FIN, :] = lin_w[h * C:(h + 1) * C, :].T
    w3[20, :] = gat_bias
    return w3


def _conv_w_permute(cw):
    """Pure layout transform: conv_w[o, ci, kc*5+kk] -> [kk*24+ci, kc*8+o]."""
    w5 = np.zeros((120, 13 * COUT), dtype=np.float32)
    for kc in range(13):
        kks = 5 if kc < 12 else 2
        for kk in range(kks):
            w5[kk * C:(kk + 1) * C, kc * COUT:(kc + 1) * COUT] = \
                cw[:, :, kc * 5 + kk].T
    return w5


# --------------------------------------------------------------------------
# device program
# --------------------------------------------------------------------------
def _build(nt_total, chunks, fin0):
    nc = bacc.Bacc(None, target_bir_lowering=False, num_swdge_queues=4)
    x_d = nc.declare_dram_parameter("x", [N, FIN], F32, isOutput=False)
    xperm_d = nc.declare_dram_parameter("xperm", [128, NBLK * FIN], F32, False)
    xs_d = nc.declare_dram_parameter("xs", [128, nt_total * FS], BF16, False)
    gidx_d = nc.declare_dram_parameter("gidx16", [128, NPT * 8], I16, False)
    amask_d = nc.declare_dram_parameter("amask", [H * C, H], F32, False)
    gam_d = nc.declare_dram_parameter("bn_gamma", [FIN], F32, False)
    bet_d = nc.declare_dram_parameter("bn_beta", [FIN], F32, False)
    lw_d = nc.declare_dram_parameter("lin_w", [H * C, FIN], F32, False)
    asc_d = nc.declare_dram_parameter("att_src", [H, C], F32, False)
    adc_d = nc.declare_dram_parameter("att_dst", [H, C], F32, False)
    cw5_d = nc.declare_dram_parameter("convw5", [120, 13 * COUT], F32, False)
    w3_d = nc.declare_dram_parameter("w3cat", [NU, C], F32, False)
    cb_d = nc.declare_dram_parameter("conv_b", [COUT], F32, False)
    out_d = nc.declare_dram_parameter("out", [GPC, COUT, TOUT], F32,
                                      isOutput=True)

    spill = nc.dram_tensor("spill", [NL + 128, EL], F32)

    with tile.TileContext(nc) as tc, ExitStack() as ctx:
        cpool = ctx.enter_context(tc.tile_pool(name="const", bufs=1))

        # ---------- persistent constants ----------
        ident = cpool.tile([128, 128], F32)
        make_identity(nc, ident[:])
        ones_col = cpool.tile([128, 1], F32)
        nc.vector.memset(ones_col[:], 1.0)
        ones_row = cpool.tile([1, 128], F32)
        nc.vector.memset(ones_row[:], 1.0)

        gidx_sb = cpool.tile([128, NPT * 8], I16)
        nc.sync.dma_start(out=gidx_sb[:], in_=gidx_d[:, :])

        accb = cpool.tile([128, NBLK * NU], F32)
        nc.vector.memset(accb[:], 0.0)
        adstb = cpool.tile([128, NBLK * H], F32)

        # W3 output projection [u=24, c'=24]: rows (h,f) = lin_w[(h,c'),f],
        # row 20 = gat_bias, rows 21:23 zero.  Pure layout of the input
        # weights (the 1/4 head-mean is folded into the attention
        # normalization), so the host supplies it pre-assembled.
        w3 = cpool.tile([NU, C], F32)
        nc.sync.dma_start(out=w3[:], in_=w3_d[:, :])

        # conv weights as 13 K-chunk stationaries [ (kk,ci) , (kc,o) ]
        wc5 = cpool.tile([120, 13 * COUT], F32)
        nc.sync.dma_start(out=wc5[:], in_=cw5_d[:, :])
        cbias = cpool.tile([COUT, 1], F32)
        nc.sync.dma_start(out=cbias[:], in_=cb_d[:, None])

        # broadcast table b56:
        # [s6(0:6) t6(6:12) wa6(12:36) wd(36:56) c(56:60) wa6s(60:84)]
        # s6/t6 = BN scale/shift (identity on the marker lane); wa6[h] =
        # [att_src-projected row (5) | 1]; wa6s = s6*wa6 so attention logits
        # come straight from the raw x stream; c[h] = sum_f t_f*wa[h,f] is
        # folded into adstb once.  The BN shift of the weighted payload is
        # applied at finalize time (numer = s*acc + t*W).
        b56 = cpool.tile([128, 84], F32)
        b84b = cpool.tile([128, 24], BF16)  # bf16 shadow of wa6s rows
        adstbb = cpool.tile([128, NBLK * H], BF16)  # bf16 shadow of adstb
        s20 = cpool.tile([128, 20], F32)   # s repeated per head
        th20 = cpool.tile([128, 20], F32)  # t repeated per head
        asrcb = cpool.tile([128, NBLK * H], F32)

        # ---------- setup phase 1: BN stats + broadcast table ----------
        with tc.tile_pool(name="setup", bufs=1) as spool, \
             tc.tile_pool(name="spsum", bufs=2, space="PSUM") as sps:
            xsb = spool.tile([128, MB * FIN], F32)
            nc.sync.dma_start(
                out=xsb[:],
                in_=x_d[:, :].rearrange("(p m) f -> p (m f)", p=128))
            tmp = spool.tile([128, MB * FIN], F32)

            # per-partition partial sums of x and x^2  -> [128, 10]
            xpart = spool.tile([128, 10], F32)
            nc.scalar.activation(out=tmp[:], in_=xsb[:], func=AF.Square)
            nc.vector.tensor_reduce(
                out=xpart[:, 0:FIN],
                in_=xsb[:].rearrange("p (m f) -> p f m", f=FIN),
                axis=mybir.AxisListType.X, op=OP.add)
            nc.vector.tensor_reduce(
                out=xpart[:, FIN:2 * FIN],
                in_=tmp[:].rearrange("p (m f) -> p f m", f=FIN),
                axis=mybir.AxisListType.X, op=OP.add)
            sums_ps = sps.tile([1, 10], F32)
            nc.tensor.matmul(out=sums_ps[:], lhsT=ones_col[:], rhs=xpart[:],
                             start=True, stop=True)

            st1 = spool.tile([1, 32], F32)
            w2 = spool.tile([1, 84], F32)
            nc.vector.memset(w2[:], 0.0)
            nc.vector.tensor_copy(out=st1[:, 0:10], in_=sums_ps[:])
            nc.vector.tensor_scalar_mul(out=st1[:, 0:5], in0=st1[:, 0:5],
                                        scalar1=1.0 / N)          # mu
            nc.vector.tensor_scalar_mul(out=st1[:, 5:10], in0=st1[:, 5:10],
                                        scalar1=1.0 / N)          # E[x^2]
            nc.vector.tensor_tensor(out=st1[:, 10:15], in0=st1[:, 0:5],
                                    in1=st1[:, 0:5], op=OP.mult)  # mu^2
            nc.vector.tensor_tensor(out=st1[:, 10:15], in0=st1[:, 5:10],
                                    in1=st1[:, 10:15], op=OP.subtract)  # var
            nc.vector.tensor_scalar_add(out=st1[:, 15:20],
                                        in0=st1[:, 10:15], scalar1=EPS)
            nc.scalar.activation(out=st1[:, 15:20], in_=st1[:, 15:20],
                                 func=AF.Sqrt)
            nc.vector.reciprocal(out=st1[:, 10:15], in_=st1[:, 15:20])  # rstd
            gsb = spool.tile([1, FIN], F32)
            bsb = spool.tile([1, FIN], F32)
            nc.sync.dma_start(out=gsb[:], in_=gam_d[None, :])
            nc.sync.dma_start(out=bsb[:], in_=bet_d[None, :])
            nc.vector.tensor_tensor(out=w2[:, 0:5], in0=gsb[:],
                                    in1=st1[:, 10:15], op=OP.mult)  # s
            nc.vector.memset(w2[:, 5:6], 1.0)                       # s[mark]=1
            nc.vector.tensor_tensor(out=st1[:, 20:25], in0=st1[:, 0:5],
                                    in1=w2[:, 0:5], op=OP.mult)     # mu*s
            nc.vector.tensor_tensor(out=w2[:, 6:11], in0=bsb[:],
                                    in1=st1[:, 20:25], op=OP.subtract)  # t
            # t[mark] = 0 (from memset)

            # wa / wd via a rank-1 matmul straight into (h-major, f) order:
            # rhs40[q, g*20+h*5+f] = a2[q, g*4+h] * lin_w[q, f]; column sums
            # (ones lhsT) give wa (g=0) and wd (g=1).
            attfs = spool.tile([H * C, 1], F32)
            attfd = spool.tile([H * C, 1], F32)
            nc.sync.dma_start(out=attfs[:],
                              in_=asc_d[:, :].rearrange("h c -> (h c)")[:, None])
            nc.sync.dma_start(out=attfd[:],
                              in_=adc_d[:, :].rearrange("h c -> (h c)")[:, None])
            amk = spool.tile([H * C, H], F32)
            nc.sync.dma_start(out=amk[:], in_=amask_d[:, :])
            a2 = spool.tile([H * C, 2 * H], F32)
            nc.vector.tensor_tensor(out=a2[:, 0:H],
                                    in0=attfs[:].to_broadcast([H * C, H]),
                                    in1=amk[:], op=OP.mult)
            nc.vector.tensor_tensor(out=a2[:, H:2 * H],
                                    in0=attfd[:].to_broadcast([H * C, H]),
                                    in1=amk[:], op=OP.mult)
            lwsb = spool.tile([H * C, FIN], F32)
            nc.sync.dma_start(out=lwsb[:], in_=lw_d[:, :])
            onesq = spool.tile([H * C, 1], F32)
            nc.vector.memset(onesq[:], 1.0)
            rhs40 = spool.tile([H * C, 2 * H * FIN], F32)
            nc.vector.tensor_tensor(
                out=rhs40[:].rearrange("q (m f) -> q m f", f=FIN),
                in0=a2[:][:, :, None].to_broadcast([H * C, 2 * H, FIN]),
                in1=lwsb[:][:, None, :].to_broadcast([H * C, 2 * H, FIN]),
                op=OP.mult)
            wps = sps.tile([1, 2 * H * FIN], F32)
            nc.tensor.matmul(out=wps[:], lhsT=onesq[:], rhs=rhs40[:],
                             start=True, stop=True)
            # wa rows into 6-wide slots with trailing 1 (marker passthrough)
            for h in range(H):
                nc.vector.tensor_copy(
                    out=w2[:, 12 + 6 * h:12 + 6 * h + 5],
                    in_=wps[:, 5 * h:5 * h + 5])
                nc.vector.memset(w2[:, 12 + 6 * h + 5:12 + 6 * h + 6], 1.0)
            nc.vector.tensor_copy(out=w2[:, 36:56], in_=wps[:, 20:40])
            # c[h] = sum_f t_f * wa[h,f]; wa6s[h] = s6 * wa6[h]
            ct5 = spool.tile([1, FIN], F32)
            for h in range(H):
                nc.vector.tensor_tensor(out=ct5[:], in0=w2[:, 6:11],
                                        in1=w2[:, 12 + 6 * h:12 + 6 * h + 5],
                                        op=OP.mult)
                nc.vector.tensor_reduce(out=w2[:, 56 + h:57 + h], in_=ct5[:],
                                        axis=mybir.AxisListType.X, op=OP.add)
                nc.vector.tensor_tensor(out=w2[:, 60 + 6 * h:66 + 6 * h],
                                        in0=w2[:, 0:6],
                                        in1=w2[:, 12 + 6 * h:18 + 6 * h],
                                        op=OP.mult)

            # broadcast [1,84] -> [128,84] with a K=1 matmul
            b56_ps = sps.tile([128, 84], F32)
            nc.tensor.matmul(out=b56_ps[:], lhsT=ones_row[:], rhs=w2[:],
                             start=True, stop=True)
            nc.vector.tensor_copy(out=b56[:], in_=b56_ps[:])
            nc.vector.tensor_copy(out=b84b[:], in_=b56[:, 60:84])
            nc.vector.tensor_copy(
                out=s20[:].rearrange("p (h f) -> p h f", f=FIN),
                in_=b56[:, 0:5][:, None, :].to_broadcast([128, H, FIN]))
            nc.vector.tensor_copy(
                out=th20[:].rearrange("p (h f) -> p h f", f=FIN),
                in_=b56[:, 6:11][:, None, :].to_broadcast([128, H, FIN]))

        # ---------- main edge loop (+ overlapped setup2 / m2) ----------
        with tc.tile_pool(name="edges", bufs=1) as epool, \
             tc.tile_pool(name="gat", bufs=4) as gpool, \
             tc.tile_pool(name="mwork", bufs=3) as mpool, \
             tc.tile_pool(name="m2", bufs=3) as m2pool, \
             tc.tile_pool(name="m2ps", bufs=2, space="PSUM") as m2ps:
            # -- setup phase 2 (overlaps the first chunks): adst/asrc in
            # rank layout, dense self-loop pass, spill pad-zeroing --
            xpb = epool.tile([128, NBLK * FIN], F32)
            nc.sync.dma_start(out=xpb[:], in_=xperm_d[:, :])
            xp_v = xpb[:].rearrange("p (m f) -> p m f", f=FIN)
            xpr = epool.tile([128, NBLK * FIN], F32)   # raw copy (payload)
            nc.vector.tensor_copy(out=xpr[:], in_=xpb[:])
            xpr_v = xpr[:].rearrange("p (m f) -> p m f", f=FIN)
            s_bc2 = b56[:, 0:FIN][:, None, :].to_broadcast([128, NBLK, FIN])
            t_bc2 = b56[:, 6:6 + FIN][:, None, :].to_broadcast([128, NBLK, FIN])
            nc.vector.tensor_tensor(out=xp_v, in0=xp_v, in1=s_bc2, op=OP.mult)
            nc.vector.tensor_tensor(out=xp_v, in0=xp_v, in1=t_bc2, op=OP.add)
            tmp2 = epool.tile([128, NBLK * FIN], F32)
            for h in range(H):
                wd_bc = b56[:, 36 + FIN * h:36 + FIN * (h + 1)]
                wd_bc = wd_bc[:, None, :].to_broadcast([128, NBLK, FIN])
                nc.vector.tensor_tensor(
                    out=tmp2[:].rearrange("p (m f) -> p m f", f=FIN),
                    in0=xp_v, in1=wd_bc, op=OP.mult)
                nc.vector.tensor_reduce(
                    out=adstb[:].rearrange("p (m h) -> p m h", h=H)[:, :, h],
                    in_=tmp2[:].rearrange("p (m f) -> p m f", f=FIN),
                    axis=mybir.AxisListType.X, op=OP.add)
            for h in range(H):
                wa_bc = b56[:, 12 + 6 * h:12 + 6 * h + FIN]
                wa_bc = wa_bc[:, None, :].to_broadcast([128, NBLK, FIN])
                nc.vector.tensor_tensor(
                    out=tmp2[:].rearrange("p (m f) -> p m f", f=FIN),
                    in0=xp_v, in1=wa_bc, op=OP.mult)
                nc.vector.tensor_reduce(
                    out=asrcb[:].rearrange("p (m h) -> p m h", h=H)[:, :, h],
                    in_=tmp2[:].rearrange("p (m f) -> p m f", f=FIN),
                    axis=mybir.AxisListType.X, op=OP.add)
            # dense self-loop pass: acc += [w_self * x_raw | w_self]
            # (payload is RAW x; the BN shift is applied at finalize time)
            zs = epool.tile([128, NBLK * H], F32)
            nc.vector.tensor_tensor(out=zs[:], in0=asrcb[:], in1=adstb[:],
                                    op=OP.add)
            nc.vector.scalar_tensor_tensor(
                out=zs[:], in0=zs[:], scalar=0.2, in1=zs[:],
                op0=OP.mult, op1=OP.max)
            nc.scalar.activation(out=zs[:], in_=zs[:], func=AF.Exp)
            # edge-stream asrc correction: adstb += c_h (after self-loop z)
            nc.vector.tensor_tensor(
                out=adstb[:].rearrange("p (m h) -> p m h", h=H),
                in0=adstb[:].rearrange("p (m h) -> p m h", h=H),
                in1=b56[:, 56:60][:, None, :].to_broadcast([128, NBLK, H]),
                op=OP.add)
            nc.vector.tensor_copy(out=adstbb[:], in_=adstb[:])
            acc_v = accb[:].rearrange("p (c u) -> p c u", u=NU)
            zs_v = zs[:].rearrange("p (m h) -> p m h", h=H)
            for h in range(H):
                nc.vector.tensor_tensor(
                    out=tmp2[:, 0:NBLK * FIN].rearrange(
                        "p (m f) -> p m f", f=FIN),
                    in0=zs_v[:, :, h:h + 1].to_broadcast([128, NBLK, FIN]),
                    in1=xpr_v, op=OP.mult)
                nc.vector.tensor_tensor(
                    out=acc_v[:, :, h * FIN:(h + 1) * FIN],
                    in0=acc_v[:, :, h * FIN:(h + 1) * FIN],
                    in1=tmp2[:, 0:NBLK * FIN].rearrange(
                        "p (m f) -> p m f", f=FIN),
                    op=OP.add)
            nc.vector.tensor_tensor(
                out=acc_v[:, :, 20:24], in0=acc_v[:, :, 20:24], in1=zs_v,
                op=OP.add)
            # zero the spill dummy rows NL.. (padded positions gather row NL)
            zrow = epool.tile([128, EL], F32)
            nc.vector.memset(zrow[:], 0.0)
            nc.sync.dma_start(out=spill[NL:NL + 128, :], in_=zrow[:])

            # normalize + project + ELU + spill a finalized rank-block range
            def emit_m2(lo, hi):
                if lo >= hi:
                    return
                nb_r = hi - lo
                rden = m2pool.tile([128, NBLK * H], F32, tag="rden")
                nc.vector.reciprocal(out=rden[:, 0:nb_r * H],
                                     in_=acc_v[:, lo:hi, 20:24])
                rd_v = rden[:, 0:nb_r * H].rearrange("p (c h) -> p c h", h=H)
                # BN shift of the raw-x payload: numer = s*acc + t*W
                # (s20/th20 are the per-(h,f) flattened s/t rows)
                ncor = m2pool.tile([128, NBLK * 20], F32, tag="ncor")
                nco_v = ncor[:, 0:nb_r * 20].rearrange(
                    "p (c h f) -> p c h f", h=H, f=FIN)
                nc.vector.tensor_copy(
                    out=nco_v,
                    in_=acc_v[:, lo:hi, 20:24][:, :, :, None].to_broadcast(
                        [128, nb_r, H, FIN]))
                nc.vector.tensor_tensor(
                    out=ncor[:, 0:nb_r * 20].rearrange(
                        "p (c u) -> p c u", u=20),
                    in0=ncor[:, 0:nb_r * 20].rearrange(
                        "p (c u) -> p c u", u=20),
                    in1=th20[:][:, None, :].to_broadcast([128, nb_r, 20]),
                    op=OP.mult)
                nc.vector.tensor_tensor(
                    out=acc_v[:, lo:hi, 0:20],
                    in0=acc_v[:, lo:hi, 0:20],
                    in1=s20[:][:, None, :].to_broadcast([128, nb_r, 20]),
                    op=OP.mult)
                nc.vector.tensor_tensor(
                    out=acc_v[:, lo:hi, 0:20],
                    in0=acc_v[:, lo:hi, 0:20],
                    in1=ncor[:, 0:nb_r * 20].rearrange(
                        "p (c u) -> p c u", u=20),
                    op=OP.add)
                # fold the 1/4 head-mean into the attention normalization
                nc.vector.scalar_tensor_tensor(
                    out=acc_v[:, lo:hi, 0:20].rearrange(
                        "p c (h f) -> p c h f", h=H),
                    in0=acc_v[:, lo:hi, 0:20].rearrange(
                        "p c (h f) -> p c h f", h=H),
                    scalar=0.25,
                    in1=rd_v[:, :, :, None].to_broadcast([128, nb_r, H, FIN]),
                    op0=OP.mult, op1=OP.mult)
                nc.vector.memset(acc_v[:, lo:hi, 20:24], 1.0)
                for b0 in range(lo, hi, 4):
                    nb = min(4, hi - b0)
                    sb4 = m2pool.tile([128, 4 * C], F32, tag="sb4")
                    for i in range(nb):
                        bb = b0 + i
                        tps = m2ps.tile([NU, 128], F32, tag="tps")
                        nc.tensor.transpose(
                            out=tps[:], in_=accb[:, bb * NU:(bb + 1) * NU],
                            identity=ident[:])
                        m1t = m2pool.tile([NU, 128], F32, tag="m1t")
                        nc.scalar.activation(out=m1t[:], in_=tps[:],
                                             func=AF.Identity)
                        ps2 = m2ps.tile([128, C], F32, tag="ps2")
                        nc.tensor.matmul(out=ps2[:], lhsT=m1t[:], rhs=w3[:],
                                         start=True, stop=True)
                        nc.scalar.activation(out=sb4[:, i * C:(i + 1) * C],
                                             in_=ps2[:], func=AF.Identity)
                    sp_view = spill[b0 * 128:(b0 + nb) * 128, 0:C]
                    nc.sync.dma_start(
                        out=sp_view.rearrange("(q p) c -> p q c", p=128),
                        in_=sb4[:, 0:nb * C].rearrange("p (q c) -> p q c", c=C))

            emit_m2(fin0, NBLK)   # blocks no round touches (usually none)

            for ci, (t0, c0, nt, fin_lo, fin_hi) in enumerate(chunks):
                xsc = gpool.tile([128, G_CH * FS], BF16, tag="xsc")
                nc.sync.dma_start(out=xsc[:, 0:nt * FS],
                                  in_=xs_d[:, t0 * FS:(t0 + nt) * FS])
                xs_v = xsc[:, 0:nt * FS].rearrange("p (t f) -> p t f", f=FS)
                # attention source logits from RAW x via wa6s = s*wa
                # (marker folds through the reduce; BN shift is in adstb)
                zt = mpool.tile([128, G_CH * H], BF16, tag="zt")
                z_v = zt[:, 0:nt * H].rearrange("p (t h) -> p t h", h=H)
                prod = mpool.tile([128, G_CH * FS], BF16, tag="prod")
                pr_v = prod[:, 0:nt * FS].rearrange("p (t f) -> p t f", f=FS)
                for h in range(H):
                    was_bc = b84b[:, 6 * h:6 * h + 6]
                    was_bc = was_bc[:, None, :].to_broadcast([128, nt, FS])
                    nc.vector.tensor_tensor(out=pr_v, in0=xs_v, in1=was_bc,
                                            op=OP.mult)
                    with nc.allow_low_precision(
                            reason="bf16 attention logits within tolerance"):
                        nc.vector.tensor_reduce(
                            out=z_v[:, :, h], in_=pr_v,
                            axis=mybir.AxisListType.X, op=OP.add)
                nc.vector.tensor_tensor(
                    out=z_v,
                    in0=z_v,
                    in1=adstbb[:].rearrange("p (m h) -> p m h", h=H)[:, c0:c0 + nt, :],
                    op=OP.add)
                wt = mpool.tile([128, G_CH * H], BF16, tag="wt")
                # leaky_relu(z, 0.2) = max(0.2*z, z)
                nc.vector.scalar_tensor_tensor(
                    out=wt[:, 0:nt * H], in0=zt[:, 0:nt * H], scalar=0.2,
                    in1=zt[:, 0:nt * H], op0=OP.mult, op1=OP.max)
                nc.scalar.activation(out=wt[:, 0:nt * H], in_=wt[:, 0:nt * H],
                                     func=AF.Exp)
                ut = mpool.tile([128, G_CH * 20], BF16, tag="ut")
                w_v = wt[:, 0:nt * H].rearrange("p (t h) -> p t h", h=H)
                nc.vector.tensor_tensor(
                    out=ut[:, 0:nt * 20].rearrange(
                        "p (t h f) -> p t h f", h=H, f=FIN),
                    in0=w_v[:, :, :, None].to_broadcast([128, nt, H, FIN]),
                    in1=xs_v[:, :, None, 0:FIN].to_broadcast([128, nt, H, FIN]),
                    op=OP.mult)
                nc.vector.tensor_tensor(
                    out=acc_v[:, c0:c0 + nt, 0:20],
                    in0=acc_v[:, c0:c0 + nt, 0:20],
                    in1=ut[:, 0:nt * 20].rearrange("p (t u) -> p t u", u=20),
                    op=OP.add)
                nc.vector.tensor_tensor(
                    out=acc_v[:, c0:c0 + nt, 20:24],
                    in0=acc_v[:, c0:c0 + nt, 20:24],
                    in1=w_v,
                    op=OP.add)
                emit_m2(fin_lo, fin_hi)

        tc.strict_bb_all_engine_barrier()   # spill DRAM RAW before unpermute

        # ---------- un-permute, ELU, transpose, conv (chunk-pipelined) ----
        # 4 super-chunks of TCH=48 position tiles (= 32 graphs = 8 conv
        # groups each); conv runs in bf16 (inputs O(1), 2e-2 tolerance).
        TCH = 48
        with tc.tile_pool(name="tail", bufs=1) as tpool, \
             tc.tile_pool(name="tl2", bufs=2) as tl2, \
             tc.tile_pool(name="tlps", bufs=2, space="PSUM") as tlps, \
             tc.tile_pool(name="tlpc", bufs=4, space="PSUM") as tlpc:
            v5 = tpool.tile([120, NPOS], BF16)
            wc5b = tpool.tile([120, 13 * COUT], BF16)
            nc.vector.tensor_copy(out=wc5b[:], in_=wc5[:])
            identb = tpool.tile([128, 128], BF16)
            nc.vector.tensor_copy(out=identb[:], in_=ident[:])
            v5v = v5[:].rearrange("q (g t) -> q g t", t=PADG)
            for ch0 in range(0, NPT, TCH):
                g2 = tl2.tile([128, TCH * EL], F32, tag="g2")
                nc.gpsimd.dma_gather(
                    g2[:].rearrange("p (t e) -> p t e", e=EL),
                    spill[:, :],
                    gidx_sb[:, ch0 * 8:(ch0 + TCH) * 8],
                    TCH * 128, TCH * 128, EL,
                    single_packet=False,
                    queue_num=(ch0 // TCH) % 4)
                g2v = g2[:].rearrange("p (t e) -> p t e", e=EL)[:, :, 0:C]
                # ELU on the strided [128, TCH, 24] view -> packed bf16
                rp = tl2.tile([128, TCH * C], F32, tag="rp")
                rp_v = rp[:].rearrange("p (t c) -> p t c", c=C)
                eb = tl2.tile([128, TCH * C], BF16, tag="eb")
                nc.vector.tensor_scalar_max(out=rp_v, in0=g2v, scalar1=0.0)
                nc.vector.tensor_scalar_min(out=g2v, in0=g2v, scalar1=0.0)
                nc.scalar.activation(out=g2v, in_=g2v, func=AF.Exp)
                nc.vector.scalar_tensor_tensor(
                    out=eb[:].rearrange("p (t c) -> p t c", c=C),
                    in0=g2v, scalar=-1.0,
                    in1=rp_v, op0=OP.add, op1=OP.add)
                for i in range(TCH):
                    blk = ch0 + i
                    tp2 = tlps.tile([C, 128], BF16, tag="tp2")
                    nc.tensor.transpose(out=tp2[:],
                                        in_=eb[:, i * C:(i + 1) * C],
                                        identity=identb[:])
                    nc.vector.tensor_copy(
                        out=v5[0:C, blk * 128:(blk + 1) * 128], in_=tp2[:])
                # shifted copies, chunk-local (conv windows never cross the
                # per-graph 192 padding, so pos+kk stays inside the chunk)
                p0 = ch0 * 128
                p1 = (ch0 + TCH) * 128
                for kk in range(1, 5):
                    nc.sync.dma_start(out=v5[kk * C:(kk + 1) * C, p0:p1 - kk],
                                      in_=v5[0:C, p0 + kk:p1])
                # conv for this chunk's 32 graphs: kc-outer over 4-group
                # PSUM batches (one ldweights per kc per batch)
                g_base = (ch0 // TCH) * 32
                for b4 in range(2):
                    pc_a = tlpc.tile([COUT, 4 * TOUT], F32, tag="pc")
                    pc_b = tlpc.tile([COUT, 4 * TOUT], F32, tag="pc")
                    pc_c = tlpc.tile([COUT, 4 * TOUT], F32, tag="pc")
                    pc_d = tlpc.tile([COUT, 4 * TOUT], F32, tag="pc")
                    pcs = [pc_a, pc_b, pc_c, pc_d]
                    for kc in range(13):
                        kdim = 120 if kc < 12 else 48
                        for gi in range(4):
                            gg = g_base // 4 + b4 * 4 + gi
                            nc.tensor.matmul(
                                out=pcs[gi][:],
                                lhsT=wc5b[0:kdim, kc * COUT:(kc + 1) * COUT],
                                rhs=v5v[0:kdim, gg * 4:(gg + 1) * 4,
                                        kc * 5:kc * 5 + TOUT],
                                start=(kc == 0), stop=(kc == 12))
                    for gi in range(4):
                        gg = g_base // 4 + b4 * 4 + gi
                        osb = tl2.tile([COUT, 4 * TOUT], F32, tag="osb")
                        # y = pc + conv_b; leaky_relu(y, .01) = max(.01y, y)
                        nc.scalar.activation(out=osb[:], in_=pcs[gi][:],
                                             func=AF.Identity, bias=cbias[:])
                        nc.vector.scalar_tensor_tensor(
                            out=osb[:], in0=osb[:], scalar=0.01,
                            in1=osb[:], op0=OP.mult, op1=OP.max)
                        nc.sync.dma_start(
                            out=out_d[gg * 4:(gg + 1) * 4, :, :].rearrange(
                                "g o t -> o g t"),
                            in_=osb[:].rearrange("o (g t) -> o g t", t=TOUT))

    nc.compile()   # Bacc: legalize sync waits (event semaphores), reg alloc
    return nc


# --------------------------------------------------------------------------
# entry point
# --------------------------------------------------------------------------
def kernel(**inputs):
    x = np.ascontiguousarray(np.asarray(inputs["x"], dtype=np.float32))
    edge_index = np.asarray(inputs["edge_index"])
    per_core, chunks, nt_total, fin0 = _host_prep(x, edge_index)

    nc = _build(nt_total, chunks, fin0)

    amask = np.zeros((H * C, H), dtype=np.float32)
    for h in range(H):
        amask[h * C:(h + 1) * C, h] = 1.0

    common = dict(
        x=x,
        amask=amask,
        bn_gamma=np.asarray(inputs["bn_gamma"], np.float32),
        bn_beta=np.asarray(inputs["bn_beta"], np.float32),
        lin_w=np.ascontiguousarray(np.asarray(inputs["lin_w"], np.float32)),
        att_src=np.ascontiguousarray(np.asarray(inputs["att_src"], np.float32)),
        att_dst=np.ascontiguousarray(np.asarray(inputs["att_dst"], np.float32)),
        convw5=_conv_w_permute(np.asarray(inputs["conv_w"], np.float32)),
        w3cat=_w3_layout(np.asarray(inputs["lin_w"], np.float32),
                         np.asarray(inputs["gat_bias"], np.float32)),
        conv_b=np.asarray(inputs["conv_b"], np.float32),
    )
    in_maps = []
    for k in range(NCORES):
        m = dict(common)
        nor = per_core[k]["node_of_rank"]
        xp = x[k * NL:(k + 1) * NL][nor]
        m["xperm"] = np.ascontiguousarray(
            xp.reshape(NBLK, 128, FIN).transpose(1, 0, 2).reshape(
                128, NBLK * FIN)).astype(np.float32)
        m["xs"] = per_core[k]["xs"]
        m["gidx16"] = per_core[k]["gidx16"]
        in_maps.append(m)

    import os
    trace = bool(os.environ.get("GAT_TRACE"))
    res = run_bass_kernel_spmd(nc, in_maps, list(range(NCORES)), trace=trace)
    global LAST_RESULT
    LAST_RESULT = res
    outs = [res.results[k]["out"] for k in range(NCORES)]
    return np.concatenate(outs, axis=0).astype(np.float32)


LAST_RESULT = None


if __name__ == "__main__":
    # smoke test with random data
    rng = np.random.default_rng(0)
    E = 3047424
    ins = dict(
        x=rng.standard_normal((N, FIN), dtype=np.float32),
        edge_index=rng.integers(0, N, size=(2, E), dtype=np.int64),
        batch=(np.arange(N, dtype=np.int64) // NPG),
        bn_gamma=np.ones(FIN, np.float32),
        bn_beta=np.zeros(FIN, np.float32),
        lin_w=rng.standard_normal((H * C, FIN), dtype=np.float32) * 0.447,
        att_src=rng.standard_normal((H, C), dtype=np.float32) * 0.1,
        att_dst=rng.standard_normal((H, C), dtype=np.float32) * 0.1,
        gat_bias=np.zeros(C, np.float32),
        conv_w=rng.standard_normal((COUT, C, KCONV), dtype=np.float32) * 0.05,
        conv_b=np.zeros(COUT, np.float32),
    )
    y = kernel(**ins)
    print(y.shape, y.dtype)


# revision 38
# speedup vs baseline: 1.0162x; 1.0162x over previous
"""GAT layer (BatchNorm -> GATConv -> head-mean -> ELU -> per-graph Conv1d)
on 8 Trainium2 NeuronCores via Bass/Tile.

Sharding: graphs (nodes + their incoming edges) are sharded across the 8
cores by destination node.  The host does index manipulation only
(sharding / sorting / padding / fancy-indexing of raw input rows); every
FLOP on tensor data happens on-device.

Per core:
  1. BN statistics over the full x (device), fused scale/shift + attention
     row vectors into a broadcast table,
  2. edges are pre-sorted by destination in-degree rank on the host so the
     per-destination segment softmax accumulates as dense "round" adds (no
     scatter); the per-edge source operand stream is the host-sharded
     raw x rows (+ a dummy marker lane), normalized and projected to
     attention logits on-device,
  3. exp / weighted payload accumulation in a [128, rank-block, 24] f32
     accumulator; finalized rank blocks are normalized, projected
     (head-mean folded), biased, ELU'd and spilled inline,
  4. un-permute node order (batched dma_gather from the spill table) and
     run the per-graph Conv1d as chunked bf16 matmuls.
"""

import sys

sys.path.insert(0, "/opt/trn_rl_repo")

import numpy as np
from contextlib import ExitStack

import concourse.bass as bass
import concourse.bacc as bacc
import concourse.tile as tile
from concourse import mybir
from concourse.masks import make_identity
from concourse.bass_utils import run_bass_kernel_spmd

F32 = mybir.dt.float32
BF16 = mybir.dt.bfloat16
I16 = mybir.dt.int16
AF = mybir.ActivationFunctionType
OP = mybir.AluOpType

N = 190464
FIN = 5
FS = 6             # streamed operand words per edge: [x(5) | marker]
H = 4
C = 24
NPG = 186          # nodes per graph
B = 1024           # graphs
NCORES = 8
GPC = B // NCORES  # 128 graphs per core
NL = N // NCORES   # 23808 local nodes per core
NBLK = NL // 128   # 186 rank blocks of 128
MB = N // 128      # 1488 nodes per partition in the flat x layout
KCONV = 62
COUT = 8
TOUT = NPG - KCONV + 1   # 125
PADG = 192               # per-graph padded length (conv shift head-room)
NPOS = GPC * PADG        # 24576 padded node positions per core
NPT = NPOS // 128        # 192 position tiles
G_CH = 192               # max edge tiles per chunk (>= NBLK: 1 chunk/round)
NU = 24                  # accum payload: 20 = w_h * xn_f, 4 = w_h
EPS = 1e-5
DUMMY_ASRC = -400.0      # exp(0.2 * (DUMMY_ASRC + adst)) ~ 1e-35, in ACT range
EL = 64                  # f32 words per spill row (legacy)
ELB = 128                # bf16 words per spill row (256B, transpose-gather)


# --------------------------------------------------------------------------
# host-side sharding / ordering (pure index manipulation)
# --------------------------------------------------------------------------
def _wrap16(vals16):
    """Wrap a linear int16 index stream for dma_gather: position i lives at
    [i % 16, i // 16]; replicate the 16-partition block to all 128."""
    cols = vals16.size // 16
    w = vals16.reshape(cols, 16).T
    return np.ascontiguousarray(np.tile(w, (8, 1)))


def _host_prep(x, edge_index):
    src_g = np.asarray(edge_index[0], dtype=np.int64)
    dst_g = np.asarray(edge_index[1], dtype=np.int64)
    cores = []
    maxdeg = 0
    for k in range(NCORES):
        lo = k * NL
        m = (dst_g >= lo) & (dst_g < lo + NL)
        # self-loops are handled by a dense on-device pass, not as edges
        es = src_g[m]
        ed = dst_g[m] - lo
        deg = np.bincount(ed, minlength=NL)
        node_of_rank = np.argsort(-deg, kind="stable")
        rank_of_node = np.empty(NL, dtype=np.int64)
        rank_of_node[node_of_rank] = np.arange(NL)
        r_e = rank_of_node[ed]
        perm = np.argsort(r_e, kind="stable")
        es_s = es[perm]
        r_s = r_e[perm]
        cnt = deg[node_of_rank]          # per-rank degree, descending
        starts = np.zeros(NL, dtype=np.int64)
        starts[1:] = np.cumsum(cnt)[:-1]
        j_s = np.arange(es_s.size, dtype=np.int64) - starts[r_s]
        maxdeg = max(maxdeg, int(cnt[0]))
        cores.append(dict(es_s=es_s, r_s=r_s, j_s=j_s, cnt=cnt,
                          rank_of_node=rank_of_node,
                          node_of_rank=node_of_rank))

    # global (SPMD-identical) round sizes: K_j = #nodes with deg > j
    kmax = np.zeros(maxdeg, dtype=np.int64)
    for c in cores:
        kj = np.searchsorted(-c["cnt"], -np.arange(maxdeg), side="left")
        kmax = np.maximum(kmax, kj)
    r_tiles = (kmax + 127) // 128            # tiles per round
    r_edges = r_tiles * 128
    round_base = np.zeros(maxdeg + 1, dtype=np.int64)
    round_base[1:] = np.cumsum(r_edges)
    e_pad = int(round_base[-1])
    nt_total = e_pad // 128

    # chunk schedule (identical across cores): (t0, c0, nt, fin_lo, fin_hi)
    # where [fin_lo, fin_hi) are the rank blocks finalized after this chunk
    # (no later round touches them -> normalize/project/spill them inline).
    chunks = []
    t0 = 0
    for j in range(maxdeg):
        rem = int(r_tiles[j])
        c0 = 0
        while rem:
            nt = min(G_CH, rem)
            last_of_round = (rem == nt)
            if last_of_round:
                hi = int(r_tiles[j])
                lo = int(r_tiles[j + 1]) if j + 1 < maxdeg else 0
            else:
                lo = hi = 0
            chunks.append((t0, c0, nt, lo, hi))
            t0 += nt
            c0 += nt
            rem -= nt
    assert t0 == nt_total
    fin0 = int(r_tiles[0]) if maxdeg else 0   # blocks never touched by rounds

    per_core = []
    for c in cores:
        stream = np.full(e_pad, N, dtype=np.int64)    # N = dummy marker
        pos = round_base[c["j_s"]] + c["r_s"]
        stream[pos] = c["es_s"]
        real = stream < N
        # per-position raw operand rows: [x(5) | marker]; dummy rows get
        # marker DUMMY_ASRC so their exp-weight underflows to ~0
        xs = np.zeros((e_pad, FS), dtype=np.float32)
        xs[real, 0:FIN] = x[stream[real]]
        xs[~real, FIN] = DUMMY_ASRC
        # position t*128+p lives at [p, t*FS : t*FS+FS]
        import ml_dtypes
        xs_sb = np.ascontiguousarray(
            xs.reshape(nt_total, 128, FS).transpose(1, 0, 2).reshape(
                128, nt_total * FS)).astype(ml_dtypes.bfloat16)

        gid = np.full(NPOS, NL, dtype=np.int64)       # NL = dummy zero row
        posg = np.arange(NPOS)
        g = posg // PADG
        s = posg % PADG
        real_g = s < NPG
        gid[real_g] = c["rank_of_node"][g[real_g] * NPG + s[real_g]]
        gidx16 = gid.astype(np.int16)

        per_core.append(dict(xs=xs_sb, gidx16=_wrap16(gidx16),
                             node_of_rank=c["node_of_rank"]))

    return per_core, chunks, nt_total, fin0


def _w3_layout(lin_w, gat_bias):
    """Pure layout: rows (h,f) = lin_w[h*24+c', f]; row 20 = gat_bias."""
    w3 = np.zeros((NU, C), dtype=np.float32)
    for h in range(H):
        w3[h * FIN:(h + 1) * FIN, :] = lin_w[h * C:(h + 1) * C, :].T
    w3[20, :] = gat_bias
    return w3


def _conv_w_permute(cw):
    """Pure layout transform: conv_w[o, ci, kc*5+kk] -> [kk*24+ci, kc*8+o]."""
    w5 = np.zeros((120, 13 * COUT), dtype=np.float32)
    for kc in range(13):
        kks = 5 if kc < 12 else 2
        for kk in range(kks):
            w5[kk * C:(kk + 1) * C, kc * COUT:(kc + 1) * COUT] = \
                cw[:, :, kc * 5 + kk].T
    return w5


# --------------------------------------------------------------------------
# device program
# --------------------------------------------------------------------------
def _build(nt_total, chunks, fin0):
    nc = bacc.Bacc(None, target_bir_lowering=False, num_swdge_queues=4)
    x_d = nc.declare_dram_parameter("x", [N, FIN], F32, isOutput=False)
    xperm_d = nc.declare_dram_parameter("xperm", [128, NBLK * FIN], F32, False)
    xs_d = nc.declare_dram_parameter("xs", [128, nt_total * FS], BF16, False)
    gidx_d = nc.declare_dram_parameter("gidx16", [128, NPT * 8], I16, False)
    amask_d = nc.declare_dram_parameter("amask", [H * C, H], F32, False)
    gam_d = nc.declare_dram_parameter("bn_gamma", [FIN], F32, False)
    bet_d = nc.declare_dram_parameter("bn_beta", [FIN], F32, False)
    lw_d = nc.declare_dram_parameter("lin_w", [H * C, FIN], F32, False)
    asc_d = nc.declare_dram_parameter("att_src", [H, C], F32, False)
    adc_d = nc.declare_dram_parameter("att_dst", [H, C], F32, False)
    cw5_d = nc.declare_dram_parameter("convw5", [120, 13 * COUT], F32, False)
    w3_d = nc.declare_dram_parameter("w3cat", [NU, C], F32, False)
    cb_d = nc.declare_dram_parameter("conv_b", [COUT], F32, False)
    out_d = nc.declare_dram_parameter("out", [GPC, COUT, TOUT], F32,
                                      isOutput=True)

    spill = nc.dram_tensor("spill", [NL + 128, EL], F32)

    with tile.TileContext(nc) as tc, ExitStack() as ctx:
        cpool = ctx.enter_context(tc.tile_pool(name="const", bufs=1))

        # ---------- persistent constants ----------
        ident = cpool.tile([128, 128], F32)
        make_identity(nc, ident[:])
        ones_col = cpool.tile([128, 1], F32)
        nc.vector.memset(ones_col[:], 1.0)
        ones_row = cpool.tile([1, 128], F32)
        nc.vector.memset(ones_row[:], 1.0)

        gidx_sb = cpool.tile([128, NPT * 8], I16)
        nc.sync.dma_start(out=gidx_sb[:], in_=gidx_d[:, :])

        accb = cpool.tile([128, NBLK * NU], F32)
        nc.vector.memset(accb[:], 0.0)
        adstb = cpool.tile([128, NBLK * H], F32)

        # W3 output projection [u=24, c'=24]: rows (h,f) = lin_w[(h,c'),f],
        # row 20 = gat_bias, rows 21:23 zero.  Pure layout of the input
        # weights (the 1/4 head-mean is folded into the attention
        # normalization), so the host supplies it pre-assembled.
        w3 = cpool.tile([NU, C], F32)
        nc.sync.dma_start(out=w3[:], in_=w3_d[:, :])

        # conv weights as 13 K-chunk stationaries [ (kk,ci) , (kc,o) ]
        wc5 = cpool.tile([120, 13 * COUT], F32)
        nc.sync.dma_start(out=wc5[:], in_=cw5_d[:, :])
        cbias = cpool.tile([COUT, 1], F32)
        nc.sync.dma_start(out=cbias[:], in_=cb_d[:, None])

        # broadcast table b56:
        # [s6(0:6) t6(6:12) wa6(12:36) wd(36:56) c(56:60) wa6s(60:84)]
        # s6/t6 = BN scale/shift (identity on the marker lane); wa6[h] =
        # [att_src-projected row (5) | 1]; wa6s = s6*wa6 so attention logits
        # come straight from the raw x stream; c[h] = sum_f t_f*wa[h,f] is
        # folded into adstb once.  The BN shift of the weighted payload is
        # applied at finalize time (numer = s*acc + t*W).
        b56 = cpool.tile([128, 84], F32)
        b84b = cpool.tile([128, 24], BF16)  # bf16 shadow of wa6s rows
        adstbb = cpool.tile([128, NBLK * H], BF16)  # bf16 shadow of adstb
        s20 = cpool.tile([128, 20], F32)   # s repeated per head
        th20 = cpool.tile([128, 20], F32)  # t repeated per head
        asrcb = cpool.tile([128, NBLK * H], F32)

        # ---------- setup phase 1: BN stats + broadcast table ----------
        with tc.tile_pool(name="setup", bufs=1) as spool, \
             tc.tile_pool(name="spsum", bufs=2, space="PSUM") as sps:
            xsb = spool.tile([128, MB * FIN], F32)
            nc.sync.dma_start(
                out=xsb[:],
                in_=x_d[:, :].rearrange("(p m) f -> p (m f)", p=128))
            tmp = spool.tile([128, MB * FIN], F32)

            # per-partition partial sums of x and x^2  -> [128, 10]
            xpart = spool.tile([128, 10], F32)
            nc.scalar.activation(out=tmp[:], in_=xsb[:], func=AF.Square)
            nc.vector.tensor_reduce(
                out=xpart[:, 0:FIN],
                in_=xsb[:].rearrange("p (m f) -> p f m", f=FIN),
                axis=mybir.AxisListType.X, op=OP.add)
            nc.vector.tensor_reduce(
                out=xpart[:, FIN:2 * FIN],
                in_=tmp[:].rearrange("p (m f) -> p f m", f=FIN),
                axis=mybir.AxisListType.X, op=OP.add)
            sums_ps = sps.tile([1, 10], F32)
            nc.tensor.matmul(out=sums_ps[:], lhsT=ones_col[:], rhs=xpart[:],
                             start=True, stop=True)

            st1 = spool.tile([1, 32], F32)
            w2 = spool.tile([1, 84], F32)
            nc.vector.memset(w2[:], 0.0)
            nc.vector.tensor_copy(out=st1[:, 0:10], in_=sums_ps[:])
            nc.vector.tensor_scalar_mul(out=st1[:, 0:5], in0=st1[:, 0:5],
                                        scalar1=1.0 / N)          # mu
            nc.vector.tensor_scalar_mul(out=st1[:, 5:10], in0=st1[:, 5:10],
                                        scalar1=1.0 / N)          # E[x^2]
            nc.vector.tensor_tensor(out=st1[:, 10:15], in0=st1[:, 0:5],
                                    in1=st1[:, 0:5], op=OP.mult)  # mu^2
            nc.vector.tensor_tensor(out=st1[:, 10:15], in0=st1[:, 5:10],
                                    in1=st1[:, 10:15], op=OP.subtract)  # var
            nc.vector.tensor_scalar_add(out=st1[:, 15:20],
                                        in0=st1[:, 10:15], scalar1=EPS)
            nc.scalar.activation(out=st1[:, 15:20], in_=st1[:, 15:20],
                                 func=AF.Sqrt)
            nc.vector.reciprocal(out=st1[:, 10:15], in_=st1[:, 15:20])  # rstd
            gsb = spool.tile([1, FIN], F32)
            bsb = spool.tile([1, FIN], F32)
            nc.sync.dma_start(out=gsb[:], in_=gam_d[None, :])
            nc.sync.dma_start(out=bsb[:], in_=bet_d[None, :])
            nc.vector.tensor_tensor(out=w2[:, 0:5], in0=gsb[:],
                                    in1=st1[:, 10:15], op=OP.mult)  # s
            nc.vector.memset(w2[:, 5:6], 1.0)                       # s[mark]=1
            nc.vector.tensor_tensor(out=st1[:, 20:25], in0=st1[:, 0:5],
                                    in1=w2[:, 0:5], op=OP.mult)     # mu*s
            nc.vector.tensor_tensor(out=w2[:, 6:11], in0=bsb[:],
                                    in1=st1[:, 20:25], op=OP.subtract)  # t
            # t[mark] = 0 (from memset)

            # wa / wd via a rank-1 matmul straight into (h-major, f) order:
            # rhs40[q, g*20+h*5+f] = a2[q, g*4+h] * lin_w[q, f]; column sums
            # (ones lhsT) give wa (g=0) and wd (g=1).
            attfs = spool.tile([H * C, 1], F32)
            attfd = spool.tile([H * C, 1], F32)
            nc.sync.dma_start(out=attfs[:],
                              in_=asc_d[:, :].rearrange("h c -> (h c)")[:, None])
            nc.sync.dma_start(out=attfd[:],
                              in_=adc_d[:, :].rearrange("h c -> (h c)")[:, None])
            amk = spool.tile([H * C, H], F32)
            nc.sync.dma_start(out=amk[:], in_=amask_d[:, :])
            a2 = spool.tile([H * C, 2 * H], F32)
            nc.vector.tensor_tensor(out=a2[:, 0:H],
                                    in0=attfs[:].to_broadcast([H * C, H]),
                                    in1=amk[:], op=OP.mult)
            nc.vector.tensor_tensor(out=a2[:, H:2 * H],
                                    in0=attfd[:].to_broadcast([H * C, H]),
                                    in1=amk[:], op=OP.mult)
            lwsb = spool.tile([H * C, FIN], F32)
            nc.sync.dma_start(out=lwsb[:], in_=lw_d[:, :])
            onesq = spool.tile([H * C, 1], F32)
            nc.vector.memset(onesq[:], 1.0)
            rhs40 = spool.tile([H * C, 2 * H * FIN], F32)
            nc.vector.tensor_tensor(
                out=rhs40[:].rearrange("q (m f) -> q m f", f=FIN),
                in0=a2[:][:, :, None].to_broadcast([H * C, 2 * H, FIN]),
                in1=lwsb[:][:, None, :].to_broadcast([H * C, 2 * H, FIN]),
                op=OP.mult)
            wps = sps.tile([1, 2 * H * FIN], F32)
            nc.tensor.matmul(out=wps[:], lhsT=onesq[:], rhs=rhs40[:],
                             start=True, stop=True)
            # wa rows into 6-wide slots with trailing 1 (marker passthrough)
            for h in range(H):
                nc.vector.tensor_copy(
                    out=w2[:, 12 + 6 * h:12 + 6 * h + 5],
                    in_=wps[:, 5 * h:5 * h + 5])
                nc.vector.memset(w2[:, 12 + 6 * h + 5:12 + 6 * h + 6], 1.0)
            nc.vector.tensor_copy(out=w2[:, 36:56], in_=wps[:, 20:40])
            # c[h] = sum_f t_f * wa[h,f]; wa6s[h] = s6 * wa6[h]
            ct5 = spool.tile([1, FIN], F32)
            for h in range(H):
                nc.vector.tensor_tensor(out=ct5[:], in0=w2[:, 6:11],
                                        in1=w2[:, 12 + 6 * h:12 + 6 * h + 5],
                                        op=OP.mult)
                nc.vector.tensor_reduce(out=w2[:, 56 + h:57 + h], in_=ct5[:],
                                        axis=mybir.AxisListType.X, op=OP.add)
                nc.vector.tensor_tensor(out=w2[:, 60 + 6 * h:66 + 6 * h],
                                        in0=w2[:, 0:6],
                                        in1=w2[:, 12 + 6 * h:18 + 6 * h],
                                        op=OP.mult)

            # broadcast [1,84] -> [128,84] with a K=1 matmul
            b56_ps = sps.tile([128, 84], F32)
            nc.tensor.matmul(out=b56_ps[:], lhsT=ones_row[:], rhs=w2[:],
                             start=True, stop=True)
            nc.vector.tensor_copy(out=b56[:], in_=b56_ps[:])
            nc.vector.tensor_copy(out=b84b[:], in_=b56[:, 60:84])
            nc.vector.tensor_copy(
                out=s20[:].rearrange("p (h f) -> p h f", f=FIN),
                in_=b56[:, 0:5][:, None, :].to_broadcast([128, H, FIN]))
            nc.vector.tensor_copy(
                out=th20[:].rearrange("p (h f) -> p h f", f=FIN),
                in_=b56[:, 6:11][:, None, :].to_broadcast([128, H, FIN]))

        # ---------- main edge loop (+ overlapped setup2 / m2) ----------
        with tc.tile_pool(name="edges", bufs=1) as epool, \
             tc.tile_pool(name="gat", bufs=3) as gpool, \
             tc.tile_pool(name="mwork", bufs=2) as mpool, \
             tc.tile_pool(name="m2", bufs=2) as m2pool, \
             tc.tile_pool(name="m2ps", bufs=2, space="PSUM") as m2ps:
            # -- setup phase 2 (overlaps the first chunks): adst/asrc in
            # rank layout, dense self-loop pass, spill pad-zeroing --
            xpb = epool.tile([128, NBLK * FIN], F32)
            nc.sync.dma_start(out=xpb[:], in_=xperm_d[:, :])
            xp_v = xpb[:].rearrange("p (m f) -> p m f", f=FIN)
            xpr = epool.tile([128, NBLK * FIN], F32)   # raw copy (payload)
            nc.vector.tensor_copy(out=xpr[:], in_=xpb[:])
            xpr_v = xpr[:].rearrange("p (m f) -> p m f", f=FIN)
            s_bc2 = b56[:, 0:FIN][:, None, :].to_broadcast([128, NBLK, FIN])
            t_bc2 = b56[:, 6:6 + FIN][:, None, :].to_broadcast([128, NBLK, FIN])
            nc.vector.tensor_tensor(out=xp_v, in0=xp_v, in1=s_bc2, op=OP.mult)
            nc.vector.tensor_tensor(out=xp_v, in0=xp_v, in1=t_bc2, op=OP.add)
            tmp2 = epool.tile([128, NBLK * FIN], F32)
            for h in range(H):
                wd_bc = b56[:, 36 + FIN * h:36 + FIN * (h + 1)]
                wd_bc = wd_bc[:, None, :].to_broadcast([128, NBLK, FIN])
                nc.vector.tensor_tensor(
                    out=tmp2[:].rearrange("p (m f) -> p m f", f=FIN),
                    in0=xp_v, in1=wd_bc, op=OP.mult)
                nc.vector.tensor_reduce(
                    out=adstb[:].rearrange("p (m h) -> p m h", h=H)[:, :, h],
                    in_=tmp2[:].rearrange("p (m f) -> p m f", f=FIN),
                    axis=mybir.AxisListType.X, op=OP.add)
            for h in range(H):
                wa_bc = b56[:, 12 + 6 * h:12 + 6 * h + FIN]
                wa_bc = wa_bc[:, None, :].to_broadcast([128, NBLK, FIN])
                nc.vector.tensor_tensor(
                    out=tmp2[:].rearrange("p (m f) -> p m f", f=FIN),
                    in0=xp_v, in1=wa_bc, op=OP.mult)
                nc.vector.tensor_reduce(
                    out=asrcb[:].rearrange("p (m h) -> p m h", h=H)[:, :, h],
                    in_=tmp2[:].rearrange("p (m f) -> p m f", f=FIN),
                    axis=mybir.AxisListType.X, op=OP.add)
            # dense self-loop pass: acc += [w_self * x_raw | w_self]
            # (payload is RAW x; the BN shift is applied at finalize time)
            zs = epool.tile([128, NBLK * H], F32)
            nc.vector.tensor_tensor(out=zs[:], in0=asrcb[:], in1=adstb[:],
                                    op=OP.add)
            nc.vector.scalar_tensor_tensor(
                out=zs[:], in0=zs[:], scalar=0.2, in1=zs[:],
                op0=OP.mult, op1=OP.max)
            nc.scalar.activation(out=zs[:], in_=zs[:], func=AF.Exp)
            # edge-stream asrc correction: adstb += c_h (after self-loop z)
            nc.vector.tensor_tensor(
                out=adstb[:].rearrange("p (m h) -> p m h", h=H),
                in0=adstb[:].rearrange("p (m h) -> p m h", h=H),
                in1=b56[:, 56:60][:, None, :].to_broadcast([128, NBLK, H]),
                op=OP.add)
            nc.vector.tensor_copy(out=adstbb[:], in_=adstb[:])
            acc_v = accb[:].rearrange("p (c u) -> p c u", u=NU)
            zs_v = zs[:].rearrange("p (m h) -> p m h", h=H)
            for h in range(H):
                nc.vector.tensor_tensor(
                    out=tmp2[:, 0:NBLK * FIN].rearrange(
                        "p (m f) -> p m f", f=FIN),
                    in0=zs_v[:, :, h:h + 1].to_broadcast([128, NBLK, FIN]),
                    in1=xpr_v, op=OP.mult)
                nc.vector.tensor_tensor(
                    out=acc_v[:, :, h * FIN:(h + 1) * FIN],
                    in0=acc_v[:, :, h * FIN:(h + 1) * FIN],
                    in1=tmp2[:, 0:NBLK * FIN].rearrange(
                        "p (m f) -> p m f", f=FIN),
                    op=OP.add)
            nc.vector.tensor_tensor(
                out=acc_v[:, :, 20:24], in0=acc_v[:, :, 20:24], in1=zs_v,
                op=OP.add)
            # zero the spill dummy rows NL.. (padded positions gather row NL)
            zrow = epool.tile([128, EL], F32)
            nc.vector.memset(zrow[:], 0.0)
            nc.sync.dma_start(out=spill[NL:NL + 128, :], in_=zrow[:])

            # normalize + project + ELU + spill a finalized rank-block range
            def emit_m2(lo, hi):
                if lo >= hi:
                    return
                nb_r = hi - lo
                rden = m2pool.tile([128, NBLK * H], F32, tag="rden")
                nc.vector.reciprocal(out=rden[:, 0:nb_r * H],
                                     in_=acc_v[:, lo:hi, 20:24])
                rd_v = rden[:, 0:nb_r * H].rearrange("p (c h) -> p c h", h=H)
                # BN shift of the raw-x payload: numer = s*acc + t*W
                # (s20/th20 are the per-(h,f) flattened s/t rows)
                ncor = m2pool.tile([128, NBLK * 20], F32, tag="ncor")
                nco_v = ncor[:, 0:nb_r * 20].rearrange(
                    "p (c h f) -> p c h f", h=H, f=FIN)
                nc.vector.tensor_copy(
                    out=nco_v,
                    in_=acc_v[:, lo:hi, 20:24][:, :, :, None].to_broadcast(
                        [128, nb_r, H, FIN]))
                nc.vector.tensor_tensor(
                    out=ncor[:, 0:nb_r * 20].rearrange(
                        "p (c u) -> p c u", u=20),
                    in0=ncor[:, 0:nb_r * 20].rearrange(
                        "p (c u) -> p c u", u=20),
                    in1=th20[:][:, None, :].to_broadcast([128, nb_r, 20]),
                    op=OP.mult)
                nc.vector.tensor_tensor(
                    out=acc_v[:, lo:hi, 0:20],
                    in0=acc_v[:, lo:hi, 0:20],
                    in1=s20[:][:, None, :].to_broadcast([128, nb_r, 20]),
                    op=OP.mult)
                nc.vector.tensor_tensor(
                    out=acc_v[:, lo:hi, 0:20],
                    in0=acc_v[:, lo:hi, 0:20],
                    in1=ncor[:, 0:nb_r * 20].rearrange(
                        "p (c u) -> p c u", u=20),
                    op=OP.add)
                # fold the 1/4 head-mean into the attention normalization
                nc.vector.scalar_tensor_tensor(
                    out=acc_v[:, lo:hi, 0:20].rearrange(
                        "p c (h f) -> p c h f", h=H),
                    in0=acc_v[:, lo:hi, 0:20].rearrange(
                        "p c (h f) -> p c h f", h=H),
                    scalar=0.25,
                    in1=rd_v[:, :, :, None].to_broadcast([128, nb_r, H, FIN]),
                    op0=OP.mult, op1=OP.mult)
                nc.vector.memset(acc_v[:, lo:hi, 20:24], 1.0)
                for b0 in range(lo, hi, 4):
                    nb = min(4, hi - b0)
                    sb4 = m2pool.tile([128, 4 * C], F32, tag="sb4")
                    for i in range(nb):
                        bb = b0 + i
                        tps = m2ps.tile([NU, 128], F32, tag="tps")
                        nc.tensor.transpose(
                            out=tps[:], in_=accb[:, bb * NU:(bb + 1) * NU],
                            identity=ident[:])
                        m1t = m2pool.tile([NU, 128], F32, tag="m1t")
                        nc.scalar.activation(out=m1t[:], in_=tps[:],
                                             func=AF.Identity)
                        ps2 = m2ps.tile([128, C], F32, tag="ps2")
                        nc.tensor.matmul(out=ps2[:], lhsT=m1t[:], rhs=w3[:],
                                         start=True, stop=True)
                        nc.scalar.activation(out=sb4[:, i * C:(i + 1) * C],
                                             in_=ps2[:], func=AF.Identity)
                    sp_view = spill[b0 * 128:(b0 + nb) * 128, 0:C]
                    nc.sync.dma_start(
                        out=sp_view.rearrange("(q p) c -> p q c", p=128),
                        in_=sb4[:, 0:nb * C].rearrange("p (q c) -> p q c", c=C))

            emit_m2(fin0, NBLK)   # blocks no round touches (usually none)

            for ci, (t0, c0, nt, fin_lo, fin_hi) in enumerate(chunks):
                xsc = gpool.tile([128, G_CH * FS], BF16, tag="xsc")
                nc.sync.dma_start(out=xsc[:, 0:nt * FS],
                                  in_=xs_d[:, t0 * FS:(t0 + nt) * FS])
                xs_v = xsc[:, 0:nt * FS].rearrange("p (t f) -> p t f", f=FS)
                # attention source logits from RAW x via wa6s = s*wa
                # (marker folds through the reduce; BN shift is in adstb)
                zt = mpool.tile([128, G_CH * H], BF16, tag="zt")
                z_v = zt[:, 0:nt * H].rearrange("p (t h) -> p t h", h=H)
                prod = mpool.tile([128, G_CH * FS], BF16, tag="prod")
                pr_v = prod[:, 0:nt * FS].rearrange("p (t f) -> p t f", f=FS)
                for h in range(H):
                    was_bc = b84b[:, 6 * h:6 * h + 6]
                    was_bc = was_bc[:, None, :].to_broadcast([128, nt, FS])
                    nc.vector.tensor_tensor(out=pr_v, in0=xs_v, in1=was_bc,
                                            op=OP.mult)
                    with nc.allow_low_precision(
                            reason="bf16 attention logits within tolerance"):
                        nc.vector.tensor_reduce(
                            out=z_v[:, :, h], in_=pr_v,
                            axis=mybir.AxisListType.X, op=OP.add)
                nc.vector.tensor_tensor(
                    out=z_v,
                    in0=z_v,
                    in1=adstbb[:].rearrange("p (m h) -> p m h", h=H)[:, c0:c0 + nt, :],
                    op=OP.add)
                wt = mpool.tile([128, G_CH * H], BF16, tag="wt")
                # leaky_relu(z, 0.2) = max(0.2*z, z)
                nc.vector.scalar_tensor_tensor(
                    out=wt[:, 0:nt * H], in0=zt[:, 0:nt * H], scalar=0.2,
                    in1=zt[:, 0:nt * H], op0=OP.mult, op1=OP.max)
                nc.scalar.activation(out=wt[:, 0:nt * H], in_=wt[:, 0:nt * H],
                                     func=AF.Exp)
                ut = mpool.tile([128, G_CH * 20], BF16, tag="ut")
                w_v = wt[:, 0:nt * H].rearrange("p (t h) -> p t h", h=H)
                nc.vector.tensor_tensor(
                    out=ut[:, 0:nt * 20].rearrange(
                        "p (t h f) -> p t h f", h=H, f=FIN),
                    in0=w_v[:, :, :, None].to_broadcast([128, nt, H, FIN]),
                    in1=xs_v[:, :, None, 0:FIN].to_broadcast([128, nt, H, FIN]),
                    op=OP.mult)
                nc.vector.tensor_tensor(
                    out=acc_v[:, c0:c0 + nt, 0:20],
                    in0=acc_v[:, c0:c0 + nt, 0:20],
                    in1=ut[:, 0:nt * 20].rearrange("p (t u) -> p t u", u=20),
                    op=OP.add)
                nc.vector.tensor_tensor(
                    out=acc_v[:, c0:c0 + nt, 20:24],
                    in0=acc_v[:, c0:c0 + nt, 20:24],
                    in1=w_v,
                    op=OP.add)
                emit_m2(fin_lo, fin_hi)

        tc.strict_bb_all_engine_barrier()   # spill DRAM RAW before unpermute

        # ---------- un-permute, ELU, transpose, conv (chunk-pipelined) ----
        # 4 super-chunks of TCH=48 position tiles (= 32 graphs = 8 conv
        # groups each); conv runs in bf16 (inputs O(1), 2e-2 tolerance).
        TCH = 48
        with tc.tile_pool(name="tail", bufs=1) as tpool, \
             tc.tile_pool(name="tl2", bufs=2) as tl2, \
             tc.tile_pool(name="tlps", bufs=2, space="PSUM") as tlps, \
             tc.tile_pool(name="tlpc", bufs=4, space="PSUM") as tlpc:
            v5 = tpool.tile([120, NPOS], BF16)
            wc5b = tpool.tile([120, 13 * COUT], BF16)
            nc.vector.tensor_copy(out=wc5b[:], in_=wc5[:])
            identb = tpool.tile([128, 128], BF16)
            nc.vector.tensor_copy(out=identb[:], in_=ident[:])
            v5v = v5[:].rearrange("q (g t) -> q g t", t=PADG)
            for ch0 in range(0, NPT, TCH):
                g2 = tl2.tile([128, TCH * EL], F32, tag="g2")
                nc.gpsimd.dma_gather(
                    g2[:].rearrange("p (t e) -> p t e", e=EL),
                    spill[:, :],
                    gidx_sb[:, ch0 * 8:(ch0 + TCH) * 8],
                    TCH * 128, TCH * 128, EL,
                    single_packet=False,
                    queue_num=(ch0 // TCH) % 4)
                g2v = g2[:].rearrange("p (t e) -> p t e", e=EL)[:, :, 0:C]
                # ELU on the strided [128, TCH, 24] view -> packed bf16
                rp = tl2.tile([128, TCH * C], F32, tag="rp")
                rp_v = rp[:].rearrange("p (t c) -> p t c", c=C)
                eb = tl2.tile([128, TCH * C], BF16, tag="eb")
                nc.vector.tensor_scalar_max(out=rp_v, in0=g2v, scalar1=0.0)
                nc.vector.tensor_scalar_min(out=g2v, in0=g2v, scalar1=0.0)
                nc.scalar.activation(out=g2v, in_=g2v, func=AF.Exp)
                nc.vector.scalar_tensor_tensor(
                    out=eb[:].rearrange("p (t c) -> p t c", c=C),
                    in0=g2v, scalar=-1.0,
                    in1=rp_v, op0=OP.add, op1=OP.add)
                for i in range(TCH):
                    blk = ch0 + i
                    tp2 = tlps.tile([C, 128], BF16, tag="tp2")
                    nc.tensor.transpose(out=tp2[:],
                                        in_=eb[:, i * C:(i + 1) * C],
                                        identity=identb[:])
                    nc.vector.tensor_copy(
                        out=v5[0:C, blk * 128:(blk + 1) * 128], in_=tp2[:])
                # shifted copies, chunk-local (conv windows never cross the
                # per-graph 192 padding, so pos+kk stays inside the chunk)
                p0 = ch0 * 128
                p1 = (ch0 + TCH) * 128
                for kk in range(1, 5):
                    nc.sync.dma_start(out=v5[kk * C:(kk + 1) * C, p0:p1 - kk],
                                      in_=v5[0:C, p0 + kk:p1])
                # conv for this chunk's 32 graphs: kc-outer over 4-group
                # PSUM batches (one ldweights per kc per batch)
                g_base = (ch0 // TCH) * 32
                for b4 in range(2):
                    pc_a = tlpc.tile([COUT, 4 * TOUT], F32, tag="pc")
                    pc_b = tlpc.tile([COUT, 4 * TOUT], F32, tag="pc")
                    pc_c = tlpc.tile([COUT, 4 * TOUT], F32, tag="pc")
                    pc_d = tlpc.tile([COUT, 4 * TOUT], F32, tag="pc")
                    pcs = [pc_a, pc_b, pc_c, pc_d]
                    for kc in range(13):
                        kdim = 120 if kc < 12 else 48
                        for gi in range(4):
                            gg = g_base // 4 + b4 * 4 + gi
                            nc.tensor.matmul(
                                out=pcs[gi][:],
                                lhsT=wc5b[0:kdim, kc * COUT:(kc + 1) * COUT],
                                rhs=v5v[0:kdim, gg * 4:(gg + 1) * 4,
                                        kc * 5:kc * 5 + TOUT],
                                start=(kc == 0), stop=(kc == 12))
                    for gi in range(4):
                        gg = g_base // 4 + b4 * 4 + gi
                        osb = tl2.tile([COUT, 4 * TOUT], F32, tag="osb")
                        # y = pc + conv_b; leaky_relu(y, .01) = max(.01y, y)
                        nc.scalar.activation(out=osb[:], in_=pcs[gi][:],
                                             func=AF.Identity, bias=cbias[:])
                        nc.vector.scalar_tensor_tensor(
                            out=osb[:], in0=osb[:], scalar=0.01,
                            in1=osb[:], op0=OP.mult, op1=OP.max)
                        nc.sync.dma_start(
                            out=out_d[gg * 4:(gg + 1) * 4, :, :].rearrange(
                                "g o t -> o g t"),
                            in_=osb[:].rearrange("o (g t) -> o g t", t=TOUT))

    nc.compile()   # Bacc: legalize sync waits (event semaphores), reg alloc
    return nc


# --------------------------------------------------------------------------
# entry point
# --------------------------------------------------------------------------
def kernel(**inputs):
    x = np.ascontiguousarray(np.asarray(inputs["x"], dtype=np.float32))
    edge_index = np.asarray(inputs["edge_index"])
    per_core, chunks, nt_total, fin0 = _host_prep(x, edge_index)

    nc = _build(nt_total, chunks, fin0)

    amask = np.zeros((H * C, H), dtype=np.float32)
    for h in range(H):
        amask[h * C:(h + 1) * C, h] = 1.0

    common = dict(
        x=x,
        amask=amask,
        bn_gamma=np.asarray(inputs["bn_gamma"], np.float32),
        bn_beta=np.asarray(inputs["bn_beta"], np.float32),
        lin_w=np.ascontiguousarray(np.asarray(inputs["lin_w"], np.float32)),
        att_src=np.ascontiguousarray(np.asarray(inputs["att_src"], np.float32)),
        att_dst=np.ascontiguousarray(np.asarray(inputs["att_dst"], np.float32)),
        convw5=_conv_w_permute(np.asarray(inputs["conv_w"], np.float32)),
        w3cat=_w3_layout(np.asarray(inputs["lin_w"], np.float32),
                         np.asarray(inputs["gat_bias"], np.float32)),
        conv_b=np.asarray(inputs["conv_b"], np.float32),
    )
    in_maps = []
    for k in range(NCORES):
        m = dict(common)
        nor = per_core[k]["node_of_rank"]
        xp = x[k * NL:(k + 1) * NL][nor]
        m["xperm"] = np.ascontiguousarray(
            xp.reshape(NBLK, 128, FIN).transpose(1, 0, 2).reshape(
                128, NBLK * FIN)).astype(np.float32)
        m["xs"] = per_core[k]["xs"]
        m["gidx16"] = per_core[k]["gidx16"]
        in_maps.append(m)

    import os
    trace = bool(os.environ.get("GAT_TRACE"))
    res = run_bass_kernel_spmd(nc, in_maps, list(range(NCORES)), trace=trace)
    global LAST_RESULT
    LAST_RESULT = res
    outs = [res.results[k]["out"] for k in range(NCORES)]
    return np.concatenate(outs, axis=0).astype(np.float32)


LAST_RESULT = None


if __name__ == "__main__":
    # smoke test with random data
    rng = np.random.default_rng(0)
    E = 3047424
    ins = dict(
        x=rng.standard_normal((N, FIN), dtype=np.float32),
        edge_index=rng.integers(0, N, size=(2, E), dtype=np.int64),
        batch=(np.arange(N, dtype=np.int64) // NPG),
        bn_gamma=np.ones(FIN, np.float32),
        bn_beta=np.zeros(FIN, np.float32),
        lin_w=rng.standard_normal((H * C, FIN), dtype=np.float32) * 0.447,
        att_src=rng.standard_normal((H, C), dtype=np.float32) * 0.1,
        att_dst=rng.standard_normal((H, C), dtype=np.float32) * 0.1,
        gat_bias=np.zeros(C, np.float32),
        conv_w=rng.standard_normal((COUT, C, KCONV), dtype=np.float32) * 0.05,
        conv_b=np.zeros(COUT, np.float32),
    )
    y = kernel(**ins)
    print(y.shape, y.dtype)
